# revision 16
# baseline (speedup 1.0000x reference)
"""Banded local attention (ATTN_WIDTH=128) with exp-before-softmax, on 8 trn2 cores.

Reference math (per batch b, row q, full S=4096 columns):
    s      = Q K^T / 8
    a      = exp(s - rowmax(s))          # full-row max m1 required
    a_mask = a * band_mask               # keep j - i in [-64, 63]
    w      = softmax(a_mask)             # over all 4096 entries incl. zeros
    out    = w V

Reformulation (validated vs reference, rel err ~2.3e-3):
  - a_mask in [0, 1] so the second softmax needs no max shift:
        w_k = e^{a_k} / (sum_band e^{a_j} + (S - nb))
  - 256-wide window per 128-row q-tile, multiplicative 0/1 mask M:
        eg    = exp(exp(sw - m1) * M)      # masked lanes -> exp(0) = 1
        denom = sum_w eg + (S - 256)
        numer = eg @ V_win + ones^T cv,  cv = sum_all V - sum_win V
        out   = numer / denom
  - m1 = max( exact row max over reordered cols [0, 2048)   [DVE reduces]
            , C + ln(sum_{cols [2048,4096)} e^{B(s-C)})/B   [ACT exp-LSE] )
    The LSE term can only OVERSHOOT the true max (by <= ln(1024)/B worst
    case, ~0.06 typical), which the output is insensitive to (~0.045 rel
    err per unit of m1 error). B=12, C=5 keeps Z = sum e^{B(s-C)} within
    [~1e-13, ~1e16], inside the Scalar engine Ln's [2^-64, 2^64] domain
    on BOTH sides (B=24 overflowed the top and corrupted m1 on HW), and
    e^{B(s-C)} in f32 range for |s| < 12.3 (observed max |s| = 7.43).

Why hybrid: ISA allows only ONE PSUM operand per DVE instruction, so the
full-row max costs 1 DVE-cycle/element from PSUM; splitting the pass
between DVE (reduce_max) and ACT (exp-accum) is the only 2x. GPSIMD has no
PSUM port, so it handles only SBUF-side work (mask multiply, denom add).

Measured on this 8-core axon trn2 setup the PE never leaves ~1GHz pstate
(627 ns for a 512-col bf16 matmul even in a 55us dependency-free stream;
fp8 DoubleRow measures no faster), so the 128 x 512-col score matmuls are
a ~80us/core hard floor; DVE/ACT/GPSIMD/DMA hide underneath it.

Two-pass structure (v8): pass A runs all score matmuls + DVE maxes + ACT
LSEs (Exp table resident throughout; dense PE stream keeps the PE at full
pstate); ONE batched Ln over all 16 tiles' Z between passes; pass B runs
window matmul + exp chain + AV matmuls + output. This removes the per-tile
Exp<->Ln ACT table thrash that cost 42us (33 x 1.28us table loads).

Sharding: 8 cores = 4 batches x 2 query-halves of 2048 rows. K columns are
reordered per core so the window of q-tile i sits at compile-time columns
[128i, 128i+256) (SPMD-uniform); pads are borrowed from the tail so every k
appears exactly once and the full-row max is order-invariant.
"""

import sys

if "/opt/trn_rl_repo" not in sys.path:
    sys.path.insert(0, "/opt/trn_rl_repo")

from contextlib import ExitStack

import numpy as np
import ml_dtypes

import concourse.bacc as bacc
import concourse.bass as bass
import concourse.tile as tile
from concourse import mybir
from concourse.bass_utils import run_bass_kernel_spmd

B, S, D = 4, 4096, 64
ATTN_WIDTH = 128
PAD = ATTN_WIDTH // 2          # 64
W = 2 * ATTN_WIDTH             # 256 window per q-tile
HALF = S // 2                  # 2048 rows per core
NT = HALF // 128               # 16 q-tiles per core
KSLICE = HALF + 2 * PAD        # 2176 window-slice columns
N_CORES = 8
F32 = mybir.dt.float32
BF16 = mybir.dt.bfloat16
F16 = mybir.dt.float16
LSE_B = 12.0
LSE_C = 5.0

_CACHE = {}


def _emit(ctx: ExitStack, tc, params):
    nc = tc.nc
    Exp = mybir.ActivationFunctionType.Exp
    Ln = mybir.ActivationFunctionType.Ln
    Copy = mybir.ActivationFunctionType.Copy
    mx = mybir.AluOpType.max
    mn = mybir.AluOpType.min
    add = mybir.AluOpType.add
    mult = mybir.AluOpType.mult
    X = mybir.AxisListType.X

    const = ctx.enter_context(tc.tile_pool(name="const", bufs=1))
    work = ctx.enter_context(tc.tile_pool(name="work", bufs=3))
    outp = ctx.enter_context(tc.tile_pool(name="outp", bufs=4))

    qtr_s = const.tile([64, HALF], BF16)
    ktr_s = const.tile([64, S], BF16)
    vsr_s = const.tile([128, (NT + 1) * 64], BF16)
    ma_s = const.tile([128, 3 * W], BF16)
    id_s = const.tile([128, 128], BF16)
    ones_s = const.tile([1, 128], F16)
    cvr_s = const.tile([1, NT * 64], F16)
    lse_bias = const.tile([128, 1], F32)
    nc.vector.memset(lse_bias[:], -LSE_B * LSE_C)
    # persistent per-core scratch: nmq holds per-tile groups of 3 negated
    # max candidates [-max(p0), -max(p1), -m_far] so pass B needs one
    # 3-wide min-reduce; zz/lnz hold the 2-per-tile LSE sums
    nmq = const.tile([128, 3 * NT], F32)
    zz_all = const.tile([128, 2 * NT], F32)
    lnz_all = const.tile([128, 2 * NT], F32)
    nmf_all = const.tile([128, 2 * NT], F32)

    # issue order = first-use order: tile 0 needs qtr[:, :128] + all ktr
    nc.sync.dma_start(qtr_s[:, 0:128], params["qtr"][:, 0:128])
    nc.sync.dma_start(ktr_s[:, 0:512], params["ktr"][:, 0:512])
    nc.sync.dma_start(ktr_s[:, 512:S], params["ktr"][:, 512:S])
    nc.sync.dma_start(qtr_s[:, 128:HALF], params["qtr"][:, 128:HALF])
    nc.sync.dma_start(ma_s[:], params["ma"][:])
    nc.sync.dma_start(id_s[:], params["idf"][:])
    nc.sync.dma_start(vsr_s[:], params["vsr"][:])
    nc.sync.dma_start(ones_s[:], params["ones"][:])
    nc.sync.dma_start(cvr_s[:], params["cvr"][:])
    out = params["out"]

    # Ln -> negated LSE max bound -> nmq[:, 3t+2], for tiles [t0, t1).
    # Batched so the Exp<->Ln ACT table swap cost is paid O(1) times, and
    # split so most of it runs during pass A instead of stalling pass B.
    def lse_fold(t0, t1):
        nc.scalar.activation(
            lnz_all[:, 2 * t0 : 2 * t1], zz_all[:, 2 * t0 : 2 * t1], Ln
        )
        nc.vector.tensor_scalar(
            out=nmf_all[:, 2 * t0 : 2 * t1], in0=lnz_all[:, 2 * t0 : 2 * t1],
            scalar1=-1.0 / LSE_B, scalar2=-LSE_C, op0=mult, op1=add,
        )
        nc.vector.tensor_tensor(
            out=nmq[:, 3 * t0 + 2 : 3 * t1 : 3],
            in0=nmf_all[:, 2 * t0 : 2 * t1 : 2],
            in1=nmf_all[:, 2 * t0 + 1 : 2 * t1 : 2], op=mn,
        )

    # ---------------- pass A: scores -> exact maxes + exp-LSE ----------------
    # one [128,1024]-pair pool filling all 8 PSUM banks: 4 pairs in flight,
    # so the PE matmul stream stays dense while DVE and ACT retire pairs in
    # parallel
    with ExitStack() as actx:
        ps_sc = actx.enter_context(tc.tile_pool(name="ps_sc", bufs=4, space="PSUM"))
        for i in range(NT):
            qtile = qtr_s[:, 128 * i : 128 * (i + 1)]
            for k in range(4):
                sc = ps_sc.tile([128, 1024], F32, tag="sc")
                for h in range(2):
                    base = 1024 * k + 512 * h
                    nc.tensor.matmul(
                        sc[:, 512 * h : 512 * h + 512], qtile,
                        ktr_s[:, base : base + 512], start=True, stop=True,
                    )
                if k < 2:
                    # pairs 0,1 (cols [0:2048)): exact negated row max on DVE
                    nc.vector.tensor_reduce(
                        nmq[:, 3 * i + k : 3 * i + k + 1], sc[:],
                        axis=X, op=mx, negate=True,
                    )
                else:
                    # pairs 2,3 (cols [2048:4096)): exp-LSE on ACT
                    scrap = work.tile([128, 1024], BF16, tag="scrap")
                    nc.scalar.activation(
                        scrap[:], sc[:], Exp, bias=lse_bias[:], scale=LSE_B,
                        accum_out=zz_all[:, 2 * i + k - 2 : 2 * i + k - 1],
                    )
            if i == 11:
                # fold tiles 0..11 while tiles 12..15 still stream on the PE
                lse_fold(0, 12)

    # ---- transition: fold the last 4 tiles only ----
    lse_fold(12, NT)

    # ---------------- pass B: window -> exp chain -> out ----------------
    with ExitStack() as bctx:
        ps_win = bctx.enter_context(tc.tile_pool(name="ps_win", bufs=4, space="PSUM"))
        ps_fv = bctx.enter_context(tc.tile_pool(name="ps_fv", bufs=4, space="PSUM"))
        for i in range(NT):
            qtile = qtr_s[:, 128 * i : 128 * (i + 1)]
            win = ps_win.tile([128, W], F32, tag="win")
            nc.tensor.matmul(
                win[:], qtile, ktr_s[:, 128 * i : 128 * i + W], start=True, stop=True
            )
            nm1 = work.tile([128, 1], F32, tag="nm1")
            nc.vector.tensor_reduce(
                nm1[:], nmq[:, 3 * i : 3 * i + 3], axis=X, op=mn
            )
            ew = work.tile([128, W], BF16, tag="ew", bufs=4)
            nc.scalar.activation(ew[:], win[:], Exp, bias=nm1[:], scale=1.0)
            msel = 0 if i == 0 else (2 if i == NT - 1 else 1)
            ewm = work.tile([128, W], BF16, tag="ewm", bufs=4)
            nc.gpsimd.tensor_tensor(
                out=ewm[:], in0=ew[:], in1=ma_s[:, W * msel : W * (msel + 1)], op=mult
            )
            eg = work.tile([128, W], BF16, tag="eg", bufs=4)
            nc.scalar.activation(eg[:], ewm[:], Exp)
            seg = work.tile([128, 1], F32, tag="seg")
            nc.vector.tensor_reduce(seg[:], eg[:], axis=X, op=add)
            den = work.tile([128, 1], F32, tag="den")
            nc.gpsimd.tensor_scalar_add(den[:], seg[:], float(S - W))
            rec = work.tile([128, 1], F32, tag="rec")
            nc.vector.reciprocal(rec[:], den[:])

            fvt = ps_fv.tile([128, 192], F32, tag="fvt")
            egt_ps = fvt[:, 0:128].bitcast(BF16)          # [128, 256] bf16
            numer = fvt[:, 128:192]
            nc.tensor.transpose(egt_ps[:, 0:128], eg[:, 0:128], id_s[:])
            nc.tensor.transpose(egt_ps[:, 128:256], eg[:, 128:256], id_s[:])
            egt = work.tile([128, W], BF16, tag="egt_sb", bufs=4)
            nc.scalar.copy(egt[:], egt_ps[:])
            nc.tensor.matmul(
                numer[:], egt[:, 0:128], vsr_s[:, 64 * i : 64 * i + 64],
                start=True, stop=False,
            )
            nc.tensor.matmul(
                numer[:], egt[:, 128:256],
                vsr_s[:, 64 * (i + 1) : 64 * (i + 1) + 64],
                start=False, stop=False,
            )
            nc.tensor.matmul(
                numer[:], ones_s[:], cvr_s[:, 64 * i : 64 * i + 64],
                start=False, stop=True,
            )

            out_sb = outp.tile([128, 64], F32, tag="out_sb")
            nc.vector.tensor_scalar_mul(out_sb[:], numer[:], rec[:])
            nc.sync.dma_start(out[128 * i : 128 * (i + 1), :], out_sb[:])


def build_program():
    nc = bacc.Bacc("TRN2", target_bir_lowering=False, debug=False)
    params = {
        "qtr": nc.declare_dram_parameter("qtr", [64, HALF], BF16, isOutput=False),
        "ktr": nc.declare_dram_parameter("ktr", [64, S], BF16, isOutput=False),
        "vsr": nc.declare_dram_parameter(
            "vsr", [128, (NT + 1) * 64], BF16, isOutput=False
        ),
        "ma": nc.declare_dram_parameter("ma", [128, 3 * W], BF16, isOutput=False),
        "idf": nc.declare_dram_parameter("idf", [128, 128], BF16, isOutput=False),
        "ones": nc.declare_dram_parameter("ones", [1, 128], F16, isOutput=False),
        "cvr": nc.declare_dram_parameter("cvr", [1, NT * 64], F16, isOutput=False),
        "out": nc.declare_dram_parameter("out", [HALF, D], F32, isOutput=True),
    }
    with tile.TileContext(nc) as tc:
        with ExitStack() as ctx:
            _emit(ctx, tc, params)
    nc.compile()
    return nc


def make_in_maps(Q, K, V):
    """Full inputs -> list of 8 per-core input dicts."""
    Q = np.ascontiguousarray(np.asarray(Q, dtype=np.float32))
    K = np.ascontiguousarray(np.asarray(K, dtype=np.float32))
    V = np.ascontiguousarray(np.asarray(V, dtype=np.float32))

    bf16 = ml_dtypes.bfloat16
    idf = np.eye(128, dtype=np.float32).astype(bf16)
    ones = np.ones((1, 128), dtype=np.float16)
    r = np.arange(128)[:, None]
    c = np.arange(W)[None, :]
    base_band = (c >= r) & (c < r + 128)

    in_maps = []
    for core in range(N_CORES):
        b, h = divmod(core, 2)
        off = h * HALF
        # fold the 1/sqrt(D) = 1/8 score scale into Q (exact: power of two)
        qt = np.ascontiguousarray(Q[b, off : off + HALF].T) * np.float32(0.125)
        qtr = qt.astype(bf16)

        # K column order: [window slice (pads borrowed from elsewhere) | rest]
        if h == 0:
            order = np.concatenate(
                [np.arange(2112, 2176), np.arange(0, 2112), np.arange(2176, S)]
            )
        else:
            order = np.concatenate(
                [np.arange(1984, S), np.arange(1920, 1984), np.arange(0, 1920)]
            )
        ktr = np.ascontiguousarray(K[b].T[:, order]).astype(bf16)

        Vpad = np.zeros((S + 2 * PAD, D), dtype=np.float32)
        Vpad[PAD : PAD + S] = V[b]
        vsl = Vpad[off : off + KSLICE]                                # [2176, 64]
        vsl_r = vsl.astype(bf16).astype(np.float32)
        vsr = np.ascontiguousarray(
            vsl_r.reshape(NT + 1, 128, D).transpose(1, 0, 2).reshape(
                128, (NT + 1) * 64
            )
        ).astype(bf16)

        # multiplicative masks: [tile0 | interior | tile15], each [128, 256]
        interior = base_band.astype(np.float32)
        m0 = interior
        m15 = interior
        if h == 0:  # global q-tile 0: need k >= 0  -> c >= 64
            m0 = (base_band & (c >= PAD)).astype(np.float32)
        else:  # global last tile: k < S -> c < 192
            m15 = (base_band & (c < 192)).astype(np.float32)
        ma = np.ascontiguousarray(
            np.concatenate([m0, interior, m15], axis=1)
        ).astype(bf16)

        # cv_i = sum_all V (exact) - sum_window V_rounded, one row per tile
        sv = V[b].sum(axis=0, dtype=np.float32)
        cv = np.zeros((1, NT * 64), dtype=np.float32)
        for i in range(NT):
            cv[0, 64 * i : 64 * i + 64] = sv - vsl_r[128 * i : 128 * i + W].sum(
                axis=0, dtype=np.float32
            )
        cvr = cv.astype(np.float16)

        in_maps.append(
            {"qtr": qtr, "ktr": ktr, "vsr": vsr, "ma": ma, "idf": idf,
             "ones": ones, "cvr": cvr}
        )
    return in_maps


def _get_program():
    if "nc" not in _CACHE:
        _CACHE["nc"] = build_program()
    return _CACHE["nc"]


def kernel(Q, K, V):
    nc = _get_program()
    in_maps = make_in_maps(Q, K, V)
    res = run_bass_kernel_spmd(nc, in_maps, list(range(N_CORES)))
    out = np.zeros((B, S, D), dtype=np.float32)
    for core in range(N_CORES):
        b, h = divmod(core, 2)
        out[b, h * HALF : (h + 1) * HALF] = res.results[core]["out"]
    return out
